# revision 44
# baseline (speedup 1.0000x reference)
"""Trainium2 Bass kernel for batched multi-head attention with post-softmax
decay mask (nn_Attention_40681930227768).

Reference (fp32):
    qkv = x @ W_qkv ; q,k,v = split(qkv)          # [B,N,H*DH]
    dots = q @ k.T * DH**-0.5                      # [B,H,N,N]
    attn = softmax(dots, -1) * decay_mask          # post-softmax mask
    out  = (attn @ v) @ W_out + b_out              # [B,N,DIM]

Strategy: pure data-parallel over batch (B=32 -> 4 batches per NeuronCore,
8 cores, no collectives).  All matmuls in bf16 with fp32 PSUM accumulation.

Per-core layout (everything "transposed" so the j/contraction axes land on
partitions):
  - host supplies x^T  [DIM, N] per batch and mask^T [N(j), N(i)] per head
  - q^T, k^T = W_{q,k}.T @ x^T      [H*DH, N]  (hd on partitions)
  - v        = x @ W_v              [N, H*DH]  (n on partitions)
  - s^T[j,i] = (k^T_h slice).T @ q^T_h         scores transposed
  - exp on ScalarE (no max-subtraction needed: dots ~ N(0,1), fp32-safe)
  - row sums of exp via ones-vector matmul on PE (contract over j=partitions)
  - p^T = exp * mask^T on VectorE (bf16)
  - outT[d,i] += v_slice.T @ p^T    accumulated over j tiles
  - outT scaled by 1/sums, stacked over heads -> [H*DH, N] = lhsT of out proj
  - y = outT.T @ W_out + b_out
"""

import numpy as np
import ml_dtypes

B, N, DIM, H, DH = 32, 1024, 512, 8, 64
SCALE = DH ** -0.5
NCORES = 8
BP = B // NCORES          # batches per core
GB = 1                    # batches per group (mask reuse granularity)
GROUPS = BP // GB
KC = DIM // 128           # contraction chunks over DIM
HD = H * DH               # 512
MC = HD // 128            # hd chunks
NT = N // 128             # n tiles
JT = N // 128             # j tiles

_cache: dict = {}


def _build_nc(bp=BP, heads=H, groups=GROUPS):
    import concourse.bacc as bacc
    import concourse.tile as tile
    import concourse.mybir as mybir

    f32 = mybir.dt.float32
    bf16 = mybir.dt.bfloat16
    Exp = mybir.ActivationFunctionType.Exp
    add_op = mybir.AluOpType.add

    gb = min(GB, bp)
    n_groups = max(1, bp // gb)

    nc = bacc.Bacc("TRN2", target_bir_lowering=False, debug=False,
                   num_devices=NCORES)

    xT_d = nc.dram_tensor("xT", [bp, DIM, N], bf16, kind="ExternalInput").ap()
    wqkv_d = nc.dram_tensor("w_qkv", [DIM, 3 * HD], bf16, kind="ExternalInput").ap()
    wout_d = nc.dram_tensor("w_out", [HD, DIM], bf16, kind="ExternalInput").ap()
    bout_d = nc.dram_tensor("b_out", [DIM], f32, kind="ExternalInput").ap()
    maskT_d = nc.dram_tensor("maskT", [heads, N, N], bf16, kind="ExternalInput").ap()
    y_d = nc.dram_tensor("y", [bp, N, DIM], f32, kind="ExternalOutput").ap()

    from contextlib import ExitStack
    with tile.TileContext(nc) as tc, ExitStack() as ctx:
        singles = ctx.enter_context(tc.tile_pool(name="singles", bufs=1))
        xt_pool = ctx.enter_context(tc.tile_pool(name="xt", bufs=2))
        qk_pool = ctx.enter_context(tc.tile_pool(name="qk", bufs=2))
        v_pool = ctx.enter_context(tc.tile_pool(name="vp", bufs=2))
        ot_pool = ctx.enter_context(tc.tile_pool(name="ot", bufs=2))
        mask_pool = ctx.enter_context(tc.tile_pool(name="mk", bufs=2))
        e_pool = ctx.enter_context(tc.tile_pool(name="ep", bufs=6))
        p_pool = ctx.enter_context(tc.tile_pool(name="pp", bufs=6))
        y_pool = ctx.enter_context(tc.tile_pool(name="yp", bufs=2))
        r_pool = ctx.enter_context(tc.tile_pool(name="rp", bufs=1))
        psum = ctx.enter_context(tc.tile_pool(name="ps", bufs=2, space="PSUM"))
        psacc = ctx.enter_context(tc.tile_pool(name="pa", bufs=2, space="PSUM"))

        # constants / weights
        wqkv_sb = singles.tile([128, KC, 3 * HD], bf16)
        nc.sync.dma_start(wqkv_sb, wqkv_d.rearrange("(kc p) f -> p kc f", p=128))
        wout_sb = singles.tile([128, MC, DIM], bf16)
        nc.sync.dma_start(wout_sb, wout_d.rearrange("(kc p) f -> p kc f", p=128))
        bout_sb = singles.tile([1, DIM], bf16)
        nc.gpsimd.dma_start(bout_sb, bout_d.rearrange("(o d) -> o d", o=1))
        bout_bc = singles.tile([128, DIM], bf16)
        nc.gpsimd.partition_broadcast(bout_bc, bout_sb)
        ones_sb = singles.tile([128, 64], bf16)
        nc.vector.memset(ones_sb, 1.0)
        # HAM warmup: ~5us of junk matmuls while the first DMAs stream, so
        # phase A starts with the PE clock already at 2.4 GHz
        warm = singles.tile([128, 512], bf16)
        nc.vector.memset(warm, 0.0)
        wps = psum.tile([128, 512], f32, tag="p2", name="warmps")
        for _ in range(14):
            nc.tensor.matmul(wps[0:64, :], ones_sb, warm,
                             start=True, stop=True, skip_group_check=True,
                             tile_position=(0, 0))

        for g in range(n_groups):
            batches = list(range(g * gb, (g + 1) * gb))
            qT, kT, vS, oT = {}, {}, {}, {}

            # ---- phase A: qkv projections for the group's batches ----
            for b in batches:
                xt = xt_pool.tile([128, KC, N], bf16, tag="xt")
                nc.sync.dma_start(xt, xT_d[b].rearrange("(kc p) n -> p kc n", p=128))
                qT[b] = qk_pool.tile([128, MC, N], bf16, tag="qT", name=f"qT{b}")
                kT[b] = qk_pool.tile([128, MC, N], bf16, tag="kT", name=f"kT{b}")
                vS[b] = v_pool.tile([128, NT, HD], bf16, tag="v", name=f"v{b}")
                oT[b] = ot_pool.tile([128, MC, N], bf16, tag="oT", name=f"oT{b}")
                # q^T and k^T: [hd, n], hd on partitions
                for m in range(2 * MC):           # 0..3 q chunks, 4..7 k chunks
                    ps = psum.tile([128, N], f32, tag="p2")
                    col = m * 128                  # column offset into W_qkv
                    for kc in range(KC):
                        lhsT = wqkv_sb[:, kc, col:col + 128]
                        for ih in range(2):
                            nc.tensor.matmul(
                                ps[:, ih * 512:(ih + 1) * 512],
                                lhsT,
                                xt[:, kc, ih * 512:(ih + 1) * 512],
                                start=(kc == 0), stop=(kc == KC - 1))
                    dst = (qT[b][:, m, :] if m < MC else kT[b][:, m - MC, :])
                    if m % 2 == 0:
                        nc.scalar.copy(dst, ps)
                    else:
                        nc.vector.tensor_copy(dst, ps)
                # v: [n, hd], n on partitions
                for nt in range(NT):
                    ps = psum.tile([128, HD], f32, tag="p2")
                    for kc in range(KC):
                        nc.tensor.matmul(
                            ps, xt[:, kc, nt * 128:(nt + 1) * 128],
                            wqkv_sb[:, kc, 2 * HD:3 * HD],
                            start=(kc == 0), stop=(kc == KC - 1))
                    if nt % 2 == 0:
                        nc.scalar.copy(vS[b][:, nt, :], ps)
                    else:
                        nc.vector.tensor_copy(vS[b][:, nt, :], ps)

            # ---- phase B: attention per head-pair (heads 2c / 2c+1 live on
            # opposite partition halves -> concurrent row/col tile groups) ----
            for c in range(heads // 2):
                hA, hB = 2 * c, 2 * c + 1
                mk = mask_pool.tile([128, 2, JT, N], bf16, tag="mask")
                nc.sync.dma_start(
                    mk, maskT_d[hA:hA + 2].rearrange("h (jt p) i -> p h jt i", p=128))
                for b in batches:
                    qA = qT[b][0:64, c, :]
                    qB = qT[b][64:128, c, :]
                    # rows 0-63: 64 copies of the exp row-sums (ones matmul);
                    # rows 64-127: the PV accumulation. One tile per head.
                    tA = psacc.tile([128, N], f32, tag="acc", name="tA")
                    tB = psacc.tile([128, N], f32, tag="acc", name="tB")

                    def ones_pv(jt, eA, eB, pA, pB):
                        # one jt behind the scores/exp/mask stage, so these
                        # PE ops never head-of-line block on ACT/DVE.
                        # All matmuls in 64x64 tiling mode; K=128 ops split
                        # into two K=64 halves on row tiles T0/T8 (ones) and
                        # T2/T10 (PV), accumulating into the same psum rows.
                        vA = vS[b][:, jt, hA * 64:(hA + 1) * 64]
                        vB = vS[b][:, jt, hB * 64:(hB + 1) * 64]
                        st, sp = (jt == 0), (jt == JT - 1)
                        for tX, eX, vX, pX in ((tA, eA, vA, pA),
                                               (tB, eB, vB, pB)):
                            for ih in range(2):
                                s = slice(ih * 512, (ih + 1) * 512)
                                nc.tensor.matmul(tX[0:64, s], ones_sb, eX[:, s],
                                                 start=st, stop=sp,
                                                 skip_group_check=True,
                                                 tile_position=(0, 0))
                            for ih in range(2):
                                s = slice(ih * 512, (ih + 1) * 512)
                                nc.tensor.matmul(tX[64:128, s], vX, pX[:, s],
                                                 start=st, stop=sp,
                                                 skip_group_check=True,
                                                 tile_position=(0, 64))

                    pending = None
                    for jt in range(JT):
                        scA = psum.tile([128, N], f32, tag="p2", name="scA")
                        scB = psum.tile([128, N], f32, tag="p2", name="scB")
                        kA = kT[b][0:64, c, jt * 128:(jt + 1) * 128]
                        kB = kT[b][64:128, c, jt * 128:(jt + 1) * 128]
                        for ih in range(2):
                            s = slice(ih * 512, (ih + 1) * 512)
                            nc.tensor.matmul(scA[:, s], kA, qA[:, s],
                                             start=True, stop=True,
                                             tile_position=(0, 0))
                        for ih in range(2):
                            s = slice(ih * 512, (ih + 1) * 512)
                            nc.tensor.matmul(scB[:, s], kB, qB[:, s],
                                             start=True, stop=True,
                                             tile_position=(64, 0))
                        eA = e_pool.tile([128, N], bf16, tag="e", name="eA")
                        nc.scalar.activation(eA, scA, Exp, scale=SCALE)
                        eB = e_pool.tile([128, N], bf16, tag="e", name="eB")
                        nc.scalar.activation(eB, scB, Exp, scale=SCALE)
                        pA = p_pool.tile([128, N], bf16, tag="p", name="pA")
                        nc.vector.tensor_mul(pA, eA, mk[:, 0, jt, :])
                        pB = p_pool.tile([128, N], bf16, tag="p", name="pB")
                        nc.vector.tensor_mul(pB, eB, mk[:, 1, jt, :])
                        if pending is not None:
                            ones_pv(*pending)
                        pending = (jt, eA, eB, pA, pB)
                    ones_pv(*pending)
                    # normalize: recip of sums (psum row 0), broadcast via
                    # gpsimd (base-0 out), scale PV rows straight from psum
                    recA = r_pool.tile([1, N], f32, tag="recA")
                    recB = r_pool.tile([1, N], f32, tag="recB")
                    nc.vector.reciprocal_approx_fast(recA, tA[0:1, :])
                    nc.vector.reciprocal_approx_fast(recB, tB[0:1, :])
                    rbA = r_pool.tile([64, N], f32, tag="rbA")
                    rbB = r_pool.tile([64, N], f32, tag="rbB")
                    nc.gpsimd.partition_broadcast(rbA, recA)
                    nc.gpsimd.partition_broadcast(rbB, recB)
                    nc.vector.tensor_mul(oT[b][0:64, c, :], tA[64:128, :], rbA)
                    nc.vector.tensor_mul(oT[b][64:128, c, :], tB[64:128, :], rbB)

                    # phase C interleaved: project batch b right after its
                    # last head-pair so the tail overlaps remaining phase B
                    if c == heads // 2 - 1:
                        for nt in range(NT):
                            ps = psum.tile([128, DIM], f32, tag="p2")
                            for kc in range(MC):
                                nc.tensor.matmul(
                                    ps, oT[b][:, kc, nt * 128:(nt + 1) * 128],
                                    wout_sb[:, kc, :],
                                    start=(kc == 0), stop=(kc == MC - 1))
                            ys = y_pool.tile([128, DIM], f32, tag="y")
                            nc.vector.tensor_tensor(ys, ps, bout_bc, add_op)
                            nc.sync.dma_start(
                                y_d[b, nt * 128:(nt + 1) * 128, :], ys)

    nc.compile()
    return nc


def _prep_inputs(x, W_qkv, W_out, b_out, decay_mask, bp=BP, heads=H):
    bf16 = ml_dtypes.bfloat16
    xT = np.ascontiguousarray(np.transpose(x, (0, 2, 1))).astype(bf16)
    maskT = np.ascontiguousarray(
        np.transpose(decay_mask[0], (0, 2, 1))).astype(bf16)
    wq = np.ascontiguousarray(W_qkv).astype(bf16)
    wo = np.ascontiguousarray(W_out).astype(bf16)
    bo = np.ascontiguousarray(b_out).astype(np.float32)
    ncores = x.shape[0] // bp
    in_maps = []
    for c in range(ncores):
        in_maps.append({
            "xT": xT[c * bp:(c + 1) * bp],
            "w_qkv": wq,
            "w_out": wo,
            "b_out": bo,
            "maskT": maskT,
        })
    return in_maps


def kernel(x, W_qkv, W_out, b_out, decay_mask, trace=False):
    from concourse.bass_utils import run_bass_kernel_spmd

    if "nc" not in _cache:
        _cache["nc"] = _build_nc()
    nc = _cache["nc"]

    in_maps = _prep_inputs(x, W_qkv, W_out, b_out, decay_mask)
    res = run_bass_kernel_spmd(nc, in_maps, core_ids=list(range(NCORES)),
                               trace=trace)
    _cache["last_result"] = res
    y = np.concatenate([res.results[c]["y"] for c in range(NCORES)], axis=0)
    return y


# revision 45
# speedup vs baseline: 1.0301x; 1.0301x over previous
"""Trainium2 Bass kernel for batched multi-head attention with post-softmax
decay mask (nn_Attention_40681930227768).

Reference (fp32):
    qkv = x @ W_qkv ; q,k,v = split(qkv)          # [B,N,H*DH]
    dots = q @ k.T * DH**-0.5                      # [B,H,N,N]
    attn = softmax(dots, -1) * decay_mask          # post-softmax mask
    out  = (attn @ v) @ W_out + b_out              # [B,N,DIM]

Strategy: pure data-parallel over batch (B=32 -> 4 batches per NeuronCore,
8 cores, no collectives).  All matmuls in bf16 with fp32 PSUM accumulation.

Per-core layout (everything "transposed" so the j/contraction axes land on
partitions):
  - host supplies x^T  [DIM, N] per batch and mask^T [N(j), N(i)] per head
  - q^T, k^T = W_{q,k}.T @ x^T      [H*DH, N]  (hd on partitions)
  - v        = x @ W_v              [N, H*DH]  (n on partitions)
  - s^T[j,i] = (k^T_h slice).T @ q^T_h         scores transposed
  - exp on ScalarE (no max-subtraction needed: dots ~ N(0,1), fp32-safe)
  - row sums of exp via ones-vector matmul on PE (contract over j=partitions)
  - p^T = exp * mask^T on VectorE (bf16)
  - outT[d,i] += v_slice.T @ p^T    accumulated over j tiles
  - outT scaled by 1/sums, stacked over heads -> [H*DH, N] = lhsT of out proj
  - y = outT.T @ W_out + b_out
"""

import numpy as np
import ml_dtypes

B, N, DIM, H, DH = 32, 1024, 512, 8, 64
SCALE = DH ** -0.5
NCORES = 8
BP = B // NCORES          # batches per core
GB = 2                    # batches per group (mask reuse granularity)
GROUPS = BP // GB
KC = DIM // 128           # contraction chunks over DIM
HD = H * DH               # 512
MC = HD // 128            # hd chunks
NT = N // 128             # n tiles
JT = N // 128             # j tiles

_cache: dict = {}


def _build_nc(bp=BP, heads=H, groups=GROUPS):
    import concourse.bacc as bacc
    import concourse.tile as tile
    import concourse.mybir as mybir

    f32 = mybir.dt.float32
    bf16 = mybir.dt.bfloat16
    Exp = mybir.ActivationFunctionType.Exp
    add_op = mybir.AluOpType.add

    gb = min(GB, bp)
    n_groups = max(1, bp // gb)

    nc = bacc.Bacc("TRN2", target_bir_lowering=False, debug=False,
                   num_devices=NCORES)

    xT_d = nc.dram_tensor("xT", [bp, DIM, N], bf16, kind="ExternalInput").ap()
    wqkv_d = nc.dram_tensor("w_qkv", [DIM, 3 * HD], bf16, kind="ExternalInput").ap()
    wout_d = nc.dram_tensor("w_out", [HD, DIM], bf16, kind="ExternalInput").ap()
    bout_d = nc.dram_tensor("b_out", [DIM], f32, kind="ExternalInput").ap()
    maskT_d = nc.dram_tensor("maskT", [heads, N, N], bf16, kind="ExternalInput").ap()
    y_d = nc.dram_tensor("y", [bp, N, DIM], f32, kind="ExternalOutput").ap()

    from contextlib import ExitStack
    with tile.TileContext(nc) as tc, ExitStack() as ctx:
        singles = ctx.enter_context(tc.tile_pool(name="singles", bufs=1))
        xt_pool = ctx.enter_context(tc.tile_pool(name="xt", bufs=2))
        qk_pool = ctx.enter_context(tc.tile_pool(name="qk", bufs=2))
        v_pool = ctx.enter_context(tc.tile_pool(name="vp", bufs=2))
        ot_pool = ctx.enter_context(tc.tile_pool(name="ot", bufs=2))
        mask_pool = ctx.enter_context(tc.tile_pool(name="mk", bufs=2))
        e_pool = ctx.enter_context(tc.tile_pool(name="ep", bufs=6))
        p_pool = ctx.enter_context(tc.tile_pool(name="pp", bufs=6))
        y_pool = ctx.enter_context(tc.tile_pool(name="yp", bufs=2))
        r_pool = ctx.enter_context(tc.tile_pool(name="rp", bufs=1))
        psum = ctx.enter_context(tc.tile_pool(name="ps", bufs=2, space="PSUM"))
        psacc = ctx.enter_context(tc.tile_pool(name="pa", bufs=2, space="PSUM"))

        # constants / weights
        wqkv_sb = singles.tile([128, KC, 3 * HD], bf16)
        nc.sync.dma_start(wqkv_sb, wqkv_d.rearrange("(kc p) f -> p kc f", p=128))
        wout_sb = singles.tile([128, MC, DIM], bf16)
        nc.sync.dma_start(wout_sb, wout_d.rearrange("(kc p) f -> p kc f", p=128))
        bout_sb = singles.tile([1, DIM], bf16)
        nc.gpsimd.dma_start(bout_sb, bout_d.rearrange("(o d) -> o d", o=1))
        bout_bc = singles.tile([128, DIM], bf16)
        nc.gpsimd.partition_broadcast(bout_bc, bout_sb)
        ones_sb = singles.tile([128, 64], bf16)
        nc.vector.memset(ones_sb, 1.0)
        # HAM warmup: ~5us of junk matmuls while the first DMAs stream, so
        # phase A starts with the PE clock already at 2.4 GHz
        warm = singles.tile([128, 512], bf16)
        nc.vector.memset(warm, 0.0)
        wps = psum.tile([128, 512], f32, tag="p2", name="warmps")
        for _ in range(14):
            nc.tensor.matmul(wps[0:64, :], ones_sb, warm,
                             start=True, stop=True, skip_group_check=True,
                             tile_position=(0, 0))

        for g in range(n_groups):
            batches = list(range(g * gb, (g + 1) * gb))
            qT, kT, vS, oT = {}, {}, {}, {}

            # ---- phase A: qkv projections for the group's batches ----
            for b in batches:
                xt = xt_pool.tile([128, KC, N], bf16, tag="xt")
                nc.sync.dma_start(xt, xT_d[b].rearrange("(kc p) n -> p kc n", p=128))
                qT[b] = qk_pool.tile([128, MC, N], bf16, tag="qT", name=f"qT{b}")
                kT[b] = qk_pool.tile([128, MC, N], bf16, tag="kT", name=f"kT{b}")
                vS[b] = v_pool.tile([128, NT, HD], bf16, tag="v", name=f"v{b}")
                oT[b] = ot_pool.tile([128, MC, N], bf16, tag="oT", name=f"oT{b}")
                # q^T and k^T: [hd, n], hd on partitions
                for m in range(2 * MC):           # 0..3 q chunks, 4..7 k chunks
                    ps = psum.tile([128, N], f32, tag="p2")
                    col = m * 128                  # column offset into W_qkv
                    for kc in range(KC):
                        lhsT = wqkv_sb[:, kc, col:col + 128]
                        for ih in range(2):
                            nc.tensor.matmul(
                                ps[:, ih * 512:(ih + 1) * 512],
                                lhsT,
                                xt[:, kc, ih * 512:(ih + 1) * 512],
                                start=(kc == 0), stop=(kc == KC - 1))
                    dst = (qT[b][:, m, :] if m < MC else kT[b][:, m - MC, :])
                    if m % 2 == 0:
                        nc.scalar.copy(dst, ps)
                    else:
                        nc.vector.tensor_copy(dst, ps)
                # v: [n, hd], n on partitions
                for nt in range(NT):
                    ps = psum.tile([128, HD], f32, tag="p2")
                    for kc in range(KC):
                        nc.tensor.matmul(
                            ps, xt[:, kc, nt * 128:(nt + 1) * 128],
                            wqkv_sb[:, kc, 2 * HD:3 * HD],
                            start=(kc == 0), stop=(kc == KC - 1))
                    if nt % 2 == 0:
                        nc.scalar.copy(vS[b][:, nt, :], ps)
                    else:
                        nc.vector.tensor_copy(vS[b][:, nt, :], ps)

            # ---- phase B: attention per head-pair (heads 2c / 2c+1 live on
            # opposite partition halves -> concurrent row/col tile groups) ----
            for c in range(heads // 2):
                hA, hB = 2 * c, 2 * c + 1
                mk = mask_pool.tile([128, 2, JT, N], bf16, tag="mask")
                nc.sync.dma_start(
                    mk, maskT_d[hA:hA + 2].rearrange("h (jt p) i -> p h jt i", p=128))
                for b in batches:
                    qA = qT[b][0:64, c, :]
                    qB = qT[b][64:128, c, :]
                    # rows 0-63: 64 copies of the exp row-sums (ones matmul);
                    # rows 64-127: the PV accumulation. One tile per head.
                    tA = psacc.tile([128, N], f32, tag="acc", name="tA")
                    tB = psacc.tile([128, N], f32, tag="acc", name="tB")

                    def ones_pv(jt, eA, eB, pA, pB):
                        # one jt behind the scores/exp/mask stage, so these
                        # PE ops never head-of-line block on ACT/DVE.
                        # All matmuls in 64x64 tiling mode; K=128 ops split
                        # into two K=64 halves on row tiles T0/T8 (ones) and
                        # T2/T10 (PV), accumulating into the same psum rows.
                        vA = vS[b][:, jt, hA * 64:(hA + 1) * 64]
                        vB = vS[b][:, jt, hB * 64:(hB + 1) * 64]
                        st, sp = (jt == 0), (jt == JT - 1)
                        for tX, eX, vX, pX in ((tA, eA, vA, pA),
                                               (tB, eB, vB, pB)):
                            for ih in range(2):
                                s = slice(ih * 512, (ih + 1) * 512)
                                nc.tensor.matmul(tX[0:64, s], ones_sb, eX[:, s],
                                                 start=st, stop=sp,
                                                 skip_group_check=True,
                                                 tile_position=(0, 0))
                            for ih in range(2):
                                s = slice(ih * 512, (ih + 1) * 512)
                                nc.tensor.matmul(tX[64:128, s], vX, pX[:, s],
                                                 start=st, stop=sp,
                                                 skip_group_check=True,
                                                 tile_position=(0, 64))

                    pending = None
                    for jt in range(JT):
                        scA = psum.tile([128, N], f32, tag="p2", name="scA")
                        scB = psum.tile([128, N], f32, tag="p2", name="scB")
                        kA = kT[b][0:64, c, jt * 128:(jt + 1) * 128]
                        kB = kT[b][64:128, c, jt * 128:(jt + 1) * 128]
                        for ih in range(2):
                            s = slice(ih * 512, (ih + 1) * 512)
                            nc.tensor.matmul(scA[:, s], kA, qA[:, s],
                                             start=True, stop=True,
                                             tile_position=(0, 0))
                        for ih in range(2):
                            s = slice(ih * 512, (ih + 1) * 512)
                            nc.tensor.matmul(scB[:, s], kB, qB[:, s],
                                             start=True, stop=True,
                                             tile_position=(64, 0))
                        eA = e_pool.tile([128, N], bf16, tag="e", name="eA")
                        nc.scalar.activation(eA, scA, Exp, scale=SCALE)
                        eB = e_pool.tile([128, N], bf16, tag="e", name="eB")
                        nc.scalar.activation(eB, scB, Exp, scale=SCALE)
                        pA = p_pool.tile([128, N], bf16, tag="p", name="pA")
                        nc.vector.tensor_mul(pA, eA, mk[:, 0, jt, :])
                        pB = p_pool.tile([128, N], bf16, tag="p", name="pB")
                        nc.vector.tensor_mul(pB, eB, mk[:, 1, jt, :])
                        if pending is not None:
                            ones_pv(*pending)
                        pending = (jt, eA, eB, pA, pB)
                    ones_pv(*pending)
                    # normalize: recip of sums (psum row 0), broadcast via
                    # gpsimd (base-0 out), scale PV rows straight from psum
                    recA = r_pool.tile([1, N], f32, tag="recA")
                    recB = r_pool.tile([1, N], f32, tag="recB")
                    nc.vector.reciprocal_approx_fast(recA, tA[0:1, :])
                    nc.vector.reciprocal_approx_fast(recB, tB[0:1, :])
                    rbA = r_pool.tile([64, N], f32, tag="rbA")
                    rbB = r_pool.tile([64, N], f32, tag="rbB")
                    nc.gpsimd.partition_broadcast(rbA, recA)
                    nc.gpsimd.partition_broadcast(rbB, recB)
                    nc.vector.tensor_mul(oT[b][0:64, c, :], tA[64:128, :], rbA)
                    nc.vector.tensor_mul(oT[b][64:128, c, :], tB[64:128, :], rbB)

                    # phase C interleaved: project batch b right after its
                    # last head-pair so the tail overlaps remaining phase B
                    if c == heads // 2 - 1:
                        for nt in range(NT):
                            ps = psum.tile([128, DIM], f32, tag="p2")
                            for kc in range(MC):
                                nc.tensor.matmul(
                                    ps, oT[b][:, kc, nt * 128:(nt + 1) * 128],
                                    wout_sb[:, kc, :],
                                    start=(kc == 0), stop=(kc == MC - 1))
                            ys = y_pool.tile([128, DIM], f32, tag="y")
                            nc.vector.tensor_tensor(ys, ps, bout_bc, add_op)
                            nc.sync.dma_start(
                                y_d[b, nt * 128:(nt + 1) * 128, :], ys)

    nc.compile()
    return nc


def _prep_inputs(x, W_qkv, W_out, b_out, decay_mask, bp=BP, heads=H):
    bf16 = ml_dtypes.bfloat16
    xT = np.ascontiguousarray(np.transpose(x, (0, 2, 1))).astype(bf16)
    maskT = np.ascontiguousarray(
        np.transpose(decay_mask[0], (0, 2, 1))).astype(bf16)
    wq = np.ascontiguousarray(W_qkv).astype(bf16)
    wo = np.ascontiguousarray(W_out).astype(bf16)
    bo = np.ascontiguousarray(b_out).astype(np.float32)
    ncores = x.shape[0] // bp
    in_maps = []
    for c in range(ncores):
        in_maps.append({
            "xT": xT[c * bp:(c + 1) * bp],
            "w_qkv": wq,
            "w_out": wo,
            "b_out": bo,
            "maskT": maskT,
        })
    return in_maps


def kernel(x, W_qkv, W_out, b_out, decay_mask, trace=False):
    from concourse.bass_utils import run_bass_kernel_spmd

    if "nc" not in _cache:
        _cache["nc"] = _build_nc()
    nc = _cache["nc"]

    in_maps = _prep_inputs(x, W_qkv, W_out, b_out, decay_mask)
    res = run_bass_kernel_spmd(nc, in_maps, core_ids=list(range(NCORES)),
                               trace=trace)
    _cache["last_result"] = res
    y = np.concatenate([res.results[c]["y"] for c in range(NCORES)], axis=0)
    return y
